# revision 1
# baseline (speedup 1.0000x reference)
"""Dcls3d (learnable-position dilated conv3d) Trainium2 kernel.

Reference computes:
  K = trilinear-scatter(weight, P) -> (64, 32, 5, 5, 5)
  out = conv3d(x, K, stride 1, pad 2) + bias     x: (2,32,16,32,32) -> out: (2,64,16,32,32)

Strategy (8 cores): shard (batch n in {0,1}) x (4 chunks of 4 output d-planes).
Each core runs an implicit-GEMM direct conv:
  - input slab (zero-padded on host) replicated 4x in SBUF, w-shifted by
    delta=0..3, giving a 128-partition (delta, ic) contraction axis.
  - for each of 25 (l, j) kernel-tap pairs: one matmul contracting
    (4 w-taps x 32 ic) = 128, M=64 out-channels, N=512 outputs, accumulating
    in PSUM; the i=4 leftover tap runs as a K=32 matmul off the delta-group.
  - bias added during PSUM->SBUF copyback; one 1MB store per core.
"""

import numpy as np

import concourse.bass as bass
import concourse.bacc as bacc
import concourse.mybir as mybir
from concourse.bass_utils import run_bass_kernel_spmd
from concourse.tile import TileContext

# ---- problem constants (hardcoded per contract) ----
N, IC, D, H, W = 2, 32, 16, 32, 32
OC = 64
KC = 16
PAD = 2
DP, HP, WP = D + 2 * PAD, H + 2 * PAD, W + 2 * PAD  # 20, 36, 36
DCHUNK = 4              # output d-planes per core
DSLAB = DCHUNK + 4      # input d-planes per core (halo 2 each side)
SLABF = DSLAB * HP * WP  # 8*36*36 = 10368
XS_COLS = SLABF + 4     # slack so the delta-shifted loads stay in bounds
NTAPS_LJ = 25
OUTF = DCHUNK * H * W   # 4096 outputs per (core, oc)

_NC_CACHE = {}


def _construct_K(weight, P):
    """Exact numpy port of reference.construct_kernel for ks=(5,5,5)."""
    Pp = P + np.float32(2.0)
    Pf = np.floor(Pp)
    R = Pp - Pf
    P1, P2, P3 = Pf[0], Pf[1], Pf[2]
    R1, R2, R3 = R[0], R[1], R[2]
    g = np.arange(5, dtype=P.dtype)[:, None, None, None]
    aL = (g == P1) * (1.0 - R1) + (g == P1 + 1.0) * R1
    aJ = (g == P3) * (1.0 - R3) + (g == P3 + 1.0) * R3
    aI = (g == P2) * (1.0 - R2) + (g == P2 + 1.0) * R2
    K = np.einsum("ock,lock,jock,iock->oclji", weight, aL, aJ, aI, optimize=True)
    return np.ascontiguousarray(K.astype(np.float32))


LJ_A = [lj for lj in range(NTAPS_LJ) if lj % 2 == 0]  # col-group 0 taps
LJ_B = [lj for lj in range(NTAPS_LJ) if lj % 2 == 1]  # col-group 1 taps
ROW_PACK = False  # leftover i=4 taps spread across PE row groups


def _build_nc_packed(mm="bf16"):
    """v1: col-group packed (2 taps concurrently on PE) + row-packed i=4."""
    key = ("v1", mm, ROW_PACK)
    if key in _NC_CACHE:
        return _NC_CACHE[key]
    f32 = mybir.dt.float32
    mdt = {"f32": f32, "bf16": mybir.dt.bfloat16}[mm]
    nc = bacc.Bacc()
    xs = nc.dram_tensor("xs", [IC, XS_COLS], mdt, kind="ExternalInput")
    kta = nc.dram_tensor("kta", [128, len(LJ_A) * OC], mdt, kind="ExternalInput")
    ktb = nc.dram_tensor("ktb", [128, len(LJ_B) * OC], mdt, kind="ExternalInput")
    ktd = nc.dram_tensor("ktd", [128, 5 * OC], mdt, kind="ExternalInput")
    ktj = nc.dram_tensor("ktj", [128, OC], mdt, kind="ExternalInput")
    kt5 = nc.dram_tensor("kt5", [IC, OC], mdt, kind="ExternalInput")
    bias = nc.dram_tensor("bias", [OC, 1], f32, kind="ExternalInput")
    out = nc.dram_tensor("out", [OC, OUTF], f32, kind="ExternalOutput")

    HALF = 6 * HP * WP  # six d-planes per xrep half
    with TileContext(nc) as tc:
        with (
            tc.tile_pool(name="const", bufs=1) as cpool,
            tc.tile_pool(name="psum", bufs=8, space="PSUM") as ppool,
        ):
            kta_sb = cpool.tile([128, len(LJ_A) * OC], mdt)
            nc.sync.dma_start(out=kta_sb, in_=kta[:, :])
            ktb_sb = cpool.tile([128, len(LJ_B) * OC], mdt)
            nc.sync.dma_start(out=ktb_sb, in_=ktb[:, :])
            ktd_sb = cpool.tile([128, 5 * OC], mdt)
            nc.sync.dma_start(out=ktd_sb, in_=ktd[:, :])
            ktj_sb = cpool.tile([128, OC], mdt)
            nc.sync.dma_start(out=ktj_sb, in_=ktj[:, :])
            kt5_sb = cpool.tile([IC, OC], mdt)
            nc.sync.dma_start(out=kt5_sb, in_=kt5[:, :])
            bias_sb = cpool.tile([OC, 1], f32)
            nc.sync.dma_start(out=bias_sb, in_=bias[:, :])
            # input slab split in two halves (planes 0-5 / 2-7) so out d=0,1
            # compute starts while the second half still loads
            xrepA = cpool.tile([128, HALF], mdt)
            xrepB = cpool.tile([128, HALF], mdt)
            for dl in range(4):
                nc.sync.dma_start(
                    out=xrepA[dl * IC : (dl + 1) * IC, :], in_=xs[:, dl : dl + HALF]
                )
            for dl in range(4):
                nc.sync.dma_start(
                    out=xrepB[dl * IC : (dl + 1) * IC, :],
                    in_=xs[:, 2 * HP * WP + dl : 2 * HP * WP + dl + HALF],
                )
            obufs = [cpool.tile([OC, H * W], f32, name=f"obuf{d}") for d in range(4)]

            # d-shifted replication for the i=4 taps: partition group
            # lam holds xs shifted by lam d-planes AND +4 in w, so one
            # K=128 matmul covers taps (l=lam, j, i=4) for lam=0..3.
            DWIN = 4 * HP * WP
            xrepD = cpool.tile([128, DWIN], mdt)
            for lam in range(4):
                o = lam * HP * WP + 4
                nc.sync.dma_start(
                    out=xrepD[lam * IC : (lam + 1) * IC, :], in_=xs[:, o : o + DWIN]
                )
            # h-row (j) shifted replication for taps (l=4, j=0..3, i=4):
            # partition group mu holds planes 4..7 shifted by mu rows and +4 w
            JWIN = 5040
            xrepJ = cpool.tile([128, JWIN], mdt)
            for mu in range(4):
                o = 4 * HP * WP + mu * WP + 4
                nc.sync.dma_start(
                    out=xrepJ[mu * IC : (mu + 1) * IC, :], in_=xs[:, o : o + JWIN]
                )

            xrepA_r = xrepA.rearrange("p (r w) -> p r w", w=WP)
            xrepB_r = xrepB.rearrange("p (r w) -> p r w", w=WP)
            xrepD_r = xrepD.rearrange("p (r w) -> p r w", w=WP)
            xrepJ_r = xrepJ.rearrange("p (r w) -> p r w", w=WP)

            def tile_geom(t):
                d, h0 = divmod(t, 2)
                h0 *= 16
                xr = xrepA_r if d < 2 else xrepB_r
                dbase = 0 if d < 2 else 2
                return d, h0, xr, dbase

            # pass 1: all w-packed taps (need only xrepA/xrepB) for all 8
            # tiles -- 8 psum banks accumulate concurrently, so the PE never
            # stalls on the later xrepD/xrepJ DMAs.
            pss = []
            for t in range(8):
                d, h0, xrep_r, dbase = tile_geom(t)
                ps = ppool.tile([128, 512], f32)
                pss.append(ps)
                for s in range(len(LJ_A)):
                    for grp, ljs, ktsb in ((0, LJ_A, kta_sb), (1, LJ_B, ktb_sb)):
                        if s >= len(ljs):
                            continue
                        lj = ljs[s]
                        l, j = divmod(lj, 5)
                        r = (d + l - dbase) * HP + h0 + j
                        nc.tensor.matmul(
                            ps[grp * 64 : grp * 64 + 64, :],
                            ktsb[:, s * OC : (s + 1) * OC],
                            xrep_r[:, r : r + 16, 0:W],
                            start=(s == 0),
                            stop=False,
                            skip_group_check=True,
                            tile_position=(0, grp * 64),
                        )
            # pass 2: i=4 closers off xrepD/xrepJ + corner single + epilogue
            for t in range(8):
                d, h0, xrep_r, dbase = tile_geom(t)
                ps = pss[t]
                for j in range(5):
                    grp = j % 2
                    nc.tensor.matmul(
                        ps[grp * 64 : grp * 64 + 64, :],
                        ktd_sb[:, j * OC : (j + 1) * OC],
                        xrepD_r[:, d * HP + h0 + j : d * HP + h0 + j + 16, 0:W],
                        start=False,
                        stop=False,
                        skip_group_check=True,
                        tile_position=(0, grp * 64),
                    )
                nc.tensor.matmul(
                    ps[64:128, :],
                    ktj_sb[:, :],
                    xrepJ_r[:, d * HP + h0 : d * HP + h0 + 16, 0:W],
                    start=False,
                    stop=True,
                    skip_group_check=True,
                    tile_position=(0, 64),
                )
                r45 = (d + 4 - dbase) * HP + h0 + 4  # tap (l=4, j=4)
                nc.tensor.matmul(
                    ps[0:64, :],
                    kt5_sb[0:IC, :],
                    xrep_r[0:IC, r45 : r45 + 16, 4 : 4 + W],
                    start=False,
                    stop=True,
                    skip_group_check=True,
                    tile_position=(0, 0),
                )
                oslice = obufs[d][:, (t % 2) * 512 : (t % 2) * 512 + 512]
                nc.vector.tensor_scalar_add(out=oslice, in0=ps[0:64, :], scalar1=bias_sb)
                nc.vector.tensor_tensor(
                    out=oslice, in0=ps[64:128, :], in1=oslice,
                    op=mybir.AluOpType.add,
                )
                if t % 2 == 1:
                    nc.sync.dma_start(
                        out=out[:, d * H * W : (d + 1) * H * W], in_=obufs[d]
                    )
    nc.finalize()
    _NC_CACHE[key] = nc
    return nc


def _build_nc(mm="bf16"):
    key = ("v0", mm)
    if key in _NC_CACHE:
        return _NC_CACHE[key]
    f32 = mybir.dt.float32
    mdt = {"f32": f32, "bf16": mybir.dt.bfloat16}[mm]
    nc = bacc.Bacc()
    xs = nc.dram_tensor("xs", [IC, XS_COLS], mdt, kind="ExternalInput")
    kt = nc.dram_tensor("kt", [128, NTAPS_LJ * OC], mdt, kind="ExternalInput")
    kt4 = nc.dram_tensor("kt4", [IC, NTAPS_LJ * OC], mdt, kind="ExternalInput")
    bias = nc.dram_tensor("bias", [OC, 1], f32, kind="ExternalInput")
    out = nc.dram_tensor("out", [OC, OUTF], f32, kind="ExternalOutput")

    with TileContext(nc) as tc:
        with (
            tc.tile_pool(name="const", bufs=1) as cpool,
            tc.tile_pool(name="psum", bufs=4, space="PSUM") as ppool,
        ):
            xrep = cpool.tile([128, SLABF], mdt)
            # partition p = dl*32+ic holds xs[ic, dl : dl+SLABF] (w-shift by dl)
            for dl in range(4):
                nc.sync.dma_start(
                    out=xrep[dl * IC : (dl + 1) * IC, :], in_=xs[:, dl : dl + SLABF]
                )
            kt_sb = cpool.tile([128, NTAPS_LJ * OC], mdt)
            nc.sync.dma_start(out=kt_sb, in_=kt[:, :])
            kt4_sb = cpool.tile([IC, NTAPS_LJ * OC], mdt)
            nc.sync.dma_start(out=kt4_sb, in_=kt4[:, :])
            bias_sb = cpool.tile([OC, 1], f32)
            nc.sync.dma_start(out=bias_sb, in_=bias[:, :])
            obuf = cpool.tile([OC, OUTF], f32)

            # view xrep free dim as (row, w) where row = d*HP + h
            xrep_r = xrep.rearrange("p (r w) -> p r w", w=WP)

            for t in range(8):  # out tile: 512 outputs = 16 h-rows x 32 w
                d, h0 = divmod(t, 2)
                h0 *= 16
                ps = ppool.tile([OC, 512], f32)
                for lj in range(NTAPS_LJ):
                    l, j = divmod(lj, 5)
                    r = (d + l) * HP + h0 + j
                    rhs = xrep_r[:, r : r + 16, 0:W]
                    nc.tensor.matmul(
                        ps,
                        kt_sb[:, lj * OC : (lj + 1) * OC],
                        rhs,
                        start=(lj == 0),
                        stop=False,
                    )
                    rhs4 = xrep_r[0:IC, r : r + 16, 4 : 4 + W]
                    nc.tensor.matmul(
                        ps,
                        kt4_sb[:, lj * OC : (lj + 1) * OC],
                        rhs4,
                        start=False,
                        stop=(lj == NTAPS_LJ - 1),
                    )
                nc.vector.tensor_scalar_add(
                    out=obuf[:, t * 512 : (t + 1) * 512], in0=ps, scalar1=bias_sb
                )
            nc.sync.dma_start(out=out[:, :], in_=obuf)
    nc.finalize()
    _NC_CACHE[key] = nc
    return nc


def kernel(x, weight, P, bias, mm="bf16", ver="v1"):
    import ml_dtypes

    x = np.ascontiguousarray(np.asarray(x, dtype=np.float32))
    weight = np.asarray(weight, dtype=np.float32)
    P = np.asarray(P, dtype=np.float32)
    bias = np.asarray(bias, dtype=np.float32)
    mnp = {"f32": np.float32, "bf16": ml_dtypes.bfloat16}[mm]

    K = _construct_K(weight, P)  # (oc, ic, l, j, i)
    # lhsT layouts: partition=(i, ic), free=(l*5+j slot, oc)
    Kt = K.transpose(4, 1, 2, 3, 0)  # (i, ic, l, j, oc)
    KtF = Kt.reshape(5, IC, NTAPS_LJ, OC)
    bias_in = np.ascontiguousarray(bias.reshape(OC, 1))

    xpad = np.pad(x, ((0, 0), (0, 0), (PAD, PAD), (PAD, PAD), (PAD, PAD)))

    if ver == "v0":
        kt = np.ascontiguousarray(KtF[:4].reshape(128, NTAPS_LJ * OC).astype(mnp))
        kt4 = np.ascontiguousarray(KtF[4].reshape(IC, NTAPS_LJ * OC).astype(mnp))
        extra = {"kt": kt, "kt4": kt4}
        build = _build_nc
    else:
        kta = np.ascontiguousarray(
            KtF[:4][:, :, LJ_A, :].reshape(128, len(LJ_A) * OC).astype(mnp)
        )
        ktb = np.ascontiguousarray(
            KtF[:4][:, :, LJ_B, :].reshape(128, len(LJ_B) * OC).astype(mnp)
        )
        # ktd: partition (l, ic) for l=0..3, free (j, oc): taps (l, j, i=4)
        ktd = np.zeros((128, 5 * OC), mnp)
        for j in range(5):
            for l in range(4):
                ktd[32 * l : 32 * (l + 1), j * OC : (j + 1) * OC] = KtF[
                    4, :, l * 5 + j, :
                ].astype(mnp)
        # ktj: partition (j, ic) for j=0..3: taps (l=4, j, i=4)
        ktj = np.zeros((128, OC), mnp)
        for j in range(4):
            ktj[32 * j : 32 * (j + 1), :] = KtF[4, :, 4 * 5 + j, :].astype(mnp)
        kt5 = np.ascontiguousarray(KtF[4, :, 24, :].astype(mnp))  # (l=4,j=4,i=4)
        extra = {"kta": kta, "ktb": ktb, "ktd": ktd, "ktj": ktj, "kt5": kt5}
        build = _build_nc_packed

    in_maps = []
    for ci in range(8):
        n, dc = divmod(ci, 4)
        slab = xpad[n, :, 4 * dc : 4 * dc + DSLAB].reshape(IC, SLABF)
        xs = np.zeros((IC, XS_COLS), mnp)
        xs[:, :SLABF] = slab.astype(mnp)
        in_maps.append({"xs": xs, "bias": bias_in, **extra})

    global _last_in_maps, _last_mm, _last_build
    _last_in_maps = in_maps
    _last_mm = mm
    _last_build = build
    nc = build(mm)
    res = run_bass_kernel_spmd(nc, in_maps, core_ids=list(range(8)))

    out = np.empty((N, OC, D, H, W), np.float32)
    for ci in range(8):
        n, dc = divmod(ci, 4)
        out[n, :, 4 * dc : 4 * dc + DCHUNK] = res.results[ci]["out"].reshape(
            OC, DCHUNK, H, W
        )
    return out



# revision 21
# speedup vs baseline: 2.0492x; 2.0492x over previous
"""Dcls3d (learnable-position dilated conv3d) Trainium2 kernel.

Reference computes:
  K = trilinear-scatter(weight, P) -> (64, 32, 5, 5, 5)
  out = conv3d(x, K, stride 1, pad 2) + bias     x: (2,32,16,32,32) -> out: (2,64,16,32,32)

Strategy (8 cores): shard (batch n in {0,1}) x (4 chunks of 4 output d-planes).
Each core runs an implicit-GEMM direct conv:
  - input slab (zero-padded on host) replicated 4x in SBUF, w-shifted by
    delta=0..3, giving a 128-partition (delta, ic) contraction axis.
  - for each of 25 (l, j) kernel-tap pairs: one matmul contracting
    (4 w-taps x 32 ic) = 128, M=64 out-channels, N=512 outputs, accumulating
    in PSUM; the i=4 leftover tap runs as a K=32 matmul off the delta-group.
  - bias added during PSUM->SBUF copyback; one 1MB store per core.
"""

import numpy as np

import concourse.bass as bass
import concourse.bacc as bacc
import concourse.mybir as mybir
from concourse.bass_utils import run_bass_kernel_spmd
from concourse.tile import TileContext

# ---- problem constants (hardcoded per contract) ----
N, IC, D, H, W = 2, 32, 16, 32, 32
OC = 64
KC = 16
PAD = 2
DP, HP, WP = D + 2 * PAD, H + 2 * PAD, W + 2 * PAD  # 20, 36, 36
DCHUNK = 4              # output d-planes per core
DSLAB = DCHUNK + 4      # input d-planes per core (halo 2 each side)
SLABF = DSLAB * HP * WP  # 8*36*36 = 10368
XS_COLS = SLABF + 4     # slack so the delta-shifted loads stay in bounds
NTAPS_LJ = 25
OUTF = DCHUNK * H * W   # 4096 outputs per (core, oc)

_NC_CACHE = {}


def _construct_K(weight, P):
    """Exact numpy port of reference.construct_kernel for ks=(5,5,5)."""
    Pp = P + np.float32(2.0)
    Pf = np.floor(Pp)
    R = Pp - Pf
    P1, P2, P3 = Pf[0], Pf[1], Pf[2]
    R1, R2, R3 = R[0], R[1], R[2]
    g = np.arange(5, dtype=P.dtype)[:, None, None, None]
    aL = (g == P1) * (1.0 - R1) + (g == P1 + 1.0) * R1
    aJ = (g == P3) * (1.0 - R3) + (g == P3 + 1.0) * R3
    aI = (g == P2) * (1.0 - R2) + (g == P2 + 1.0) * R2
    K = np.einsum("ock,lock,jock,iock->oclji", weight, aL, aJ, aI, optimize=True)
    return np.ascontiguousarray(K.astype(np.float32))


LJ_A = [lj for lj in range(NTAPS_LJ) if lj % 2 == 0]  # col-group 0 taps
LJ_B = [lj for lj in range(NTAPS_LJ) if lj % 2 == 1]  # col-group 1 taps
ROW_PACK = False  # leftover i=4 taps spread across PE row groups


def _build_nc_packed(mm="bf16"):
    """v1: col-group packed (2 taps concurrently on PE) + row-packed i=4."""
    key = ("v1", mm, ROW_PACK)
    if key in _NC_CACHE:
        return _NC_CACHE[key]
    f32 = mybir.dt.float32
    mdt = {"f32": f32, "bf16": mybir.dt.bfloat16}[mm]
    nc = bacc.Bacc()
    xs = nc.dram_tensor("xs", [IC, XS_COLS], mdt, kind="ExternalInput")
    kta = nc.dram_tensor("kta", [128, len(LJ_A) * OC], mdt, kind="ExternalInput")
    ktb = nc.dram_tensor("ktb", [128, len(LJ_B) * OC], mdt, kind="ExternalInput")
    ktd = nc.dram_tensor("ktd", [128, 5 * OC], mdt, kind="ExternalInput")
    ktj = nc.dram_tensor("ktj", [128, OC], mdt, kind="ExternalInput")
    kt5 = nc.dram_tensor("kt5", [IC, OC], mdt, kind="ExternalInput")
    bias = nc.dram_tensor("bias", [OC, 1], f32, kind="ExternalInput")
    out = nc.dram_tensor("out", [OC, OUTF], f32, kind="ExternalOutput")

    HALF = 6 * HP * WP  # six d-planes per xrep half
    with TileContext(nc) as tc:
        with (
            tc.tile_pool(name="const", bufs=1) as cpool,
            tc.tile_pool(name="psum", bufs=8, space="PSUM") as ppool,
        ):
            kta_sb = cpool.tile([128, len(LJ_A) * OC], mdt)
            nc.sync.dma_start(out=kta_sb, in_=kta[:, :])
            ktb_sb = cpool.tile([128, len(LJ_B) * OC], mdt)
            nc.sync.dma_start(out=ktb_sb, in_=ktb[:, :])
            ktd_sb = cpool.tile([128, 5 * OC], mdt)
            nc.sync.dma_start(out=ktd_sb, in_=ktd[:, :])
            ktj_sb = cpool.tile([128, OC], mdt)
            nc.sync.dma_start(out=ktj_sb, in_=ktj[:, :])
            kt5_sb = cpool.tile([IC, OC], mdt)
            nc.sync.dma_start(out=kt5_sb, in_=kt5[:, :])
            bias_sb = cpool.tile([OC, 1], f32)
            nc.sync.dma_start(out=bias_sb, in_=bias[:, :])
            # input slab split in two halves (planes 0-5 / 2-7) so out d=0,1
            # compute starts while the second half still loads
            xrepA = cpool.tile([128, HALF], mdt)
            xrepB = cpool.tile([128, HALF], mdt)
            for dl in range(4):
                nc.sync.dma_start(
                    out=xrepA[dl * IC : (dl + 1) * IC, :], in_=xs[:, dl : dl + HALF]
                )
            for dl in range(4):
                nc.sync.dma_start(
                    out=xrepB[dl * IC : (dl + 1) * IC, :],
                    in_=xs[:, 2 * HP * WP + dl : 2 * HP * WP + dl + HALF],
                )
            obufs = [cpool.tile([OC, H * W], f32, name=f"obuf{d}") for d in range(4)]

            # d-shifted replication for the i=4 taps: partition group
            # lam holds xs shifted by lam d-planes AND +4 in w, so one
            # K=128 matmul covers taps (l=lam, j, i=4) for lam=0..3.
            DWIN = 4 * HP * WP
            xrepD = cpool.tile([128, DWIN], mdt)
            for lam in range(4):
                o = lam * HP * WP + 4
                nc.sync.dma_start(
                    out=xrepD[lam * IC : (lam + 1) * IC, :], in_=xs[:, o : o + DWIN]
                )
            # h-row (j) shifted replication for taps (l=4, j=0..3, i=4):
            # partition group mu holds planes 4..7 shifted by mu rows and +4 w
            JWIN = 5040
            xrepJ = cpool.tile([128, JWIN], mdt)
            for mu in range(4):
                o = 4 * HP * WP + mu * WP + 4
                nc.sync.dma_start(
                    out=xrepJ[mu * IC : (mu + 1) * IC, :], in_=xs[:, o : o + JWIN]
                )

            xrepA_r = xrepA.rearrange("p (r w) -> p r w", w=WP)
            xrepB_r = xrepB.rearrange("p (r w) -> p r w", w=WP)
            xrepD_r = xrepD.rearrange("p (r w) -> p r w", w=WP)
            xrepJ_r = xrepJ.rearrange("p (r w) -> p r w", w=WP)

            def tile_geom(t):
                d, h0 = divmod(t, 2)
                h0 *= 16
                xr = xrepA_r if d < 2 else xrepB_r
                dbase = 0 if d < 2 else 2
                return d, h0, xr, dbase

            # pass 1: all w-packed taps (need only xrepA/xrepB) for all 8
            # tiles -- 8 psum banks accumulate concurrently, so the PE never
            # stalls on the later xrepD/xrepJ DMAs.
            pss = []
            for t in range(8):
                d, h0, xrep_r, dbase = tile_geom(t)
                ps = ppool.tile([128, 512], f32)
                pss.append(ps)
                for s in range(len(LJ_A)):
                    for grp, ljs, ktsb in ((0, LJ_A, kta_sb), (1, LJ_B, ktb_sb)):
                        if s >= len(ljs):
                            continue
                        lj = ljs[s]
                        l, j = divmod(lj, 5)
                        r = (d + l - dbase) * HP + h0 + j
                        nc.tensor.matmul(
                            ps[grp * 64 : grp * 64 + 64, :],
                            ktsb[:, s * OC : (s + 1) * OC],
                            xrep_r[:, r : r + 16, 0:W],
                            start=(s == 0),
                            stop=False,
                            skip_group_check=True,
                            tile_position=(0, grp * 64),
                        )
            # pass 2: i=4 closers off xrepD/xrepJ + corner single + epilogue
            for t in range(8):
                d, h0, xrep_r, dbase = tile_geom(t)
                ps = pss[t]
                for j in range(5):
                    grp = j % 2
                    nc.tensor.matmul(
                        ps[grp * 64 : grp * 64 + 64, :],
                        ktd_sb[:, j * OC : (j + 1) * OC],
                        xrepD_r[:, d * HP + h0 + j : d * HP + h0 + j + 16, 0:W],
                        start=False,
                        stop=False,
                        skip_group_check=True,
                        tile_position=(0, grp * 64),
                    )
                nc.tensor.matmul(
                    ps[64:128, :],
                    ktj_sb[:, :],
                    xrepJ_r[:, d * HP + h0 : d * HP + h0 + 16, 0:W],
                    start=False,
                    stop=True,
                    skip_group_check=True,
                    tile_position=(0, 64),
                )
                r45 = (d + 4 - dbase) * HP + h0 + 4  # tap (l=4, j=4)
                nc.tensor.matmul(
                    ps[0:64, :],
                    kt5_sb[0:IC, :],
                    xrep_r[0:IC, r45 : r45 + 16, 4 : 4 + W],
                    start=False,
                    stop=True,
                    skip_group_check=True,
                    tile_position=(0, 0),
                )
                oslice = obufs[d][:, (t % 2) * 512 : (t % 2) * 512 + 512]
                nc.vector.tensor_scalar_add(out=oslice, in0=ps[0:64, :], scalar1=bias_sb)
                nc.vector.tensor_tensor(
                    out=oslice, in0=ps[64:128, :], in1=oslice,
                    op=mybir.AluOpType.add,
                )
                if t % 2 == 1:
                    nc.sync.dma_start(
                        out=out[:, d * H * W : (d + 1) * H * W], in_=obufs[d]
                    )
    nc.finalize()
    _NC_CACHE[key] = nc
    return nc


def _build_nc_v2(mm="bf16", warm=9, fchunks=((0, 2), (2, 5), (5, 8)),
                 bias_late=False, tail_split=0):
    """v2: plane-paired M=128 matmuls via sliding-window weight layout.

    Pair-tile = (dbase in {0,2}, h0 in {0,16}): psum[0:64] = out plane
    dbase+1, psum[64:128] = plane dbase, N = 512 (16 h-rows x 32 w).
    Main taps (i=0..3): windows (m in 0..5, j in 0..4); lhsT slides over a
    7-block [Z|l0..l4|Z] column layout so one K=128 matmul feeds both
    planes. i=4 taps: 2 windows x 5 j on a d-shifted (+4w) stack.
    152 matmuls total vs 256 in v1.
    """
    key = ("v2", mm, warm, fchunks, bias_late, tail_split)
    if key in _NC_CACHE:
        return _NC_CACHE[key]
    f32 = mybir.dt.float32
    mdt = {"f32": f32, "bf16": mybir.dt.bfloat16}[mm]
    nc = bacc.Bacc()
    xs = nc.dram_tensor("xs", [IC, XS_COLS], mdt, kind="ExternalInput")
    ktm = nc.dram_tensor("ktm", [128, 5 * 7 * OC], mdt, kind="ExternalInput")
    ktd = nc.dram_tensor("ktd", [128, 5 * 2 * 128], mdt, kind="ExternalInput")
    bias = nc.dram_tensor("bias", [128, 1], f32, kind="ExternalInput")
    out = nc.dram_tensor("out", [OC, OUTF], f32, kind="ExternalOutput")

    PL = HP * WP  # 1296 elems per padded plane
    DWIN = 5 * PL  # xd free extent (view planes 0..4 -> x planes lam..lam+4)
    with TileContext(nc) as tc:
        with (
            tc.tile_pool(name="const", bufs=1) as cpool,
            tc.tile_pool(name="psum", bufs=1, space="PSUM") as ppool,
        ):
            # PE warmup: ramp the clock to full p-state during the input DMAs
            wsb = cpool.tile([128, 512], mdt)
            nc.gpsimd.memset(wsb[:, :], 0.0)
            wps = ppool.tile([128, 512], f32)
            for _ in range(warm):
                nc.tensor.matmul(wps, wsb[:, 0:128], wsb[:, 0:512],
                                 start=True, stop=True)

            bias_sb = cpool.tile([128, 1], f32)
            if not bias_late:
                nc.sync.dma_start(out=bias_sb, in_=bias[:, :])
            ktm_sb = cpool.tile([128, 5 * 7 * OC], mdt)
            nc.sync.dma_start(out=ktm_sb, in_=ktm[:, :])
            # w-shift stack: partition (delta, ic) holds xs[ic, c+delta]
            xf = cpool.tile([128, 8 * PL], mdt)
            for p0, p1 in fchunks[:2]:
                for dl in range(4):
                    nc.sync.dma_start(
                        out=xf[dl * IC : (dl + 1) * IC, p0 * PL : p1 * PL],
                        in_=xs[:, p0 * PL + dl : p1 * PL + dl],
                    )
            if bias_late:
                nc.sync.dma_start(out=bias_sb, in_=bias[:, :])
            ktd_sb = cpool.tile([128, 5 * 2 * 128], mdt)
            nc.sync.dma_start(out=ktd_sb, in_=ktd[:, :])
            # d-shift stack for i=4: partition (lam, ic) = xs[ic, c+lam*PL+4]
            xd = cpool.tile([128, DWIN], mdt)
            for lam in range(4):
                nc.sync.dma_start(
                    out=xd[lam * IC : (lam + 1) * IC, :],
                    in_=xs[:, lam * PL + 4 : lam * PL + 4 + DWIN],
                )
            for p0, p1 in fchunks[2:]:
                for dl in range(4):
                    nc.sync.dma_start(
                        out=xf[dl * IC : (dl + 1) * IC, p0 * PL : p1 * PL],
                        in_=xs[:, p0 * PL + dl : p1 * PL + dl],
                    )

            xf_r = xf.rearrange("p (r w) -> p r w", w=WP)
            xd_r = xd.rearrange("p (r w) -> p r w", w=WP)

            # tile descriptors: (dbase, h0, nrows); the last `tail_split`
            # pair-tiles are split into two half-width psum groups so the
            # first half's epilogue+stores overlap the second half's matmuls
            pts = [(0, 0), (0, 16), (2, 0), (2, 16)]
            tiles = []
            for k, (dbase, h0) in enumerate(pts):
                if k >= len(pts) - tail_split:
                    tiles.append((dbase, h0, 8))
                    tiles.append((dbase, h0 + 8, 8))
                else:
                    tiles.append((dbase, h0, 16))
            psums = {
                t: ppool.tile([128, 32 * t[2]], f32, name=f"ps_{t[0]}_{t[1]}")
                for t in tiles
            }
            obufs = {
                t: cpool.tile([128, 32 * t[2]], f32, name=f"ob_{t[0]}_{t[1]}")
                for t in tiles
            }

            def main_mms(t):
                dbase, h0, nr = t
                ps = psums[t]
                for m in range(6):
                    for j in range(5):
                        r = (dbase + m) * HP + h0 + j
                        nc.tensor.matmul(
                            ps,
                            ktm_sb[:, j * 448 + m * 64 : j * 448 + m * 64 + 128],
                            xf_r[:, r : r + nr, 0:W],
                            start=(m == 0 and j == 0),
                            stop=False,
                        )

            def close_mms(t):
                dbase, h0, nr = t
                ps = psums[t]
                for pi, p in enumerate((0, 2)):
                    for j in range(5):
                        r = (dbase + p) * HP + h0 + j
                        nc.tensor.matmul(
                            ps,
                            ktd_sb[:, j * 256 + pi * 128 : j * 256 + pi * 128 + 128],
                            xd_r[:, r : r + nr, 0:W],
                            start=False,
                            stop=(pi == 1 and j == 4),
                        )

            def epilogue(t, q0=None, q1=None):
                dbase, h0, nr = t
                ps = psums[t]
                ob = obufs[t]
                nc.vector.tensor_scalar_add(out=ob[:, :], in0=ps[:, :],
                                            scalar1=bias_sb)
                base = dbase * H * W + h0 * W
                (q0 or nc.scalar).dma_start(out=out[:, base : base + nr * W],
                                            in_=ob[64:128, :])
                base1 = (dbase + 1) * H * W + h0 * W
                (q1 or nc.scalar).dma_start(out=out[:, base1 : base1 + nr * W],
                                            in_=ob[0:64, :])

            # compute order: mains of a dbase-pair back to back, then closers
            # (xd loads later than xf), epilogues as each psum completes
            half = len(tiles) // 2
            for gi, grp in enumerate((tiles[:half], tiles[half:])):
                for t in grp:
                    main_mms(t)
                for k, t in enumerate(grp):
                    close_mms(t)
                    last = gi == 1 and k >= len(grp) - 2
                    epilogue(t, q0=nc.sync if last else None,
                             q1=nc.scalar if last else None)
    nc.finalize()
    _NC_CACHE[key] = nc
    return nc


def _build_nc_v3(mm="bf16", warm=8, fchunks=(1, 1, 2, 2, 2), tail_split=2,
                 store_q=("scalar", "sync"), bias_pos=1):
    """v3: like v2 but shift-replication is host-side (xs4/xd4 in DRAM), so
    each chunk load is ONE DMA; weight blocks in descending-l order so psum
    halves map to ascending planes, enabling one 3D-AP store per pair-tile.
    """
    key = ("v3", mm, warm, fchunks, tail_split, store_q, bias_pos)
    if key in _NC_CACHE:
        return _NC_CACHE[key]
    f32 = mybir.dt.float32
    mdt = {"f32": f32, "bf16": mybir.dt.bfloat16}[mm]
    nc = bacc.Bacc()
    PL = HP * WP  # 1296
    DWIN = 5 * PL
    xs4 = nc.dram_tensor("xs4", [128, 8 * PL], mdt, kind="ExternalInput")
    xd4 = nc.dram_tensor("xd4", [128, DWIN], mdt, kind="ExternalInput")
    ktm = nc.dram_tensor("ktm", [128, 5 * 7 * OC], mdt, kind="ExternalInput")
    ktd = nc.dram_tensor("ktd", [128, 5 * 2 * 128], mdt, kind="ExternalInput")
    bias = nc.dram_tensor("bias", [128, 1], f32, kind="ExternalInput")
    out = nc.dram_tensor("out", [DCHUNK, OC, H * W], f32, kind="ExternalOutput")

    qmap = {"scalar": None, "sync": None, "vector": None, "gpsimd": None}
    with TileContext(nc) as tc:
        with (
            tc.tile_pool(name="const", bufs=1) as cpool,
            tc.tile_pool(name="psum", bufs=1, space="PSUM") as ppool,
        ):
            qmap = {"scalar": nc.scalar, "sync": nc.sync, "vector": nc.vector,
                    "gpsimd": nc.gpsimd}
            wsb = cpool.tile([128, 512], mdt)
            nc.gpsimd.memset(wsb[:, :], 0.0)
            wps = ppool.tile([128, 512], f32)
            for _ in range(warm):
                nc.tensor.matmul(wps, wsb[:, 0:128], wsb[:, 0:512],
                                 start=True, stop=True)

            ktm_sb = cpool.tile([128, 5 * 7 * OC], mdt)
            nc.sync.dma_start(out=ktm_sb, in_=ktm[:, :])
            bias_sb = cpool.tile([128, 1], f32)
            xf = cpool.tile([128, 8 * PL], mdt)
            p0 = 0
            for k, span in enumerate(fchunks):
                p1 = p0 + span
                nc.sync.dma_start(out=xf[:, p0 * PL : p1 * PL],
                                  in_=xs4[:, p0 * PL : p1 * PL])
                p0 = p1
                if k + 1 == bias_pos:
                    # tiny DMA placed here: the tile scheduler's sem target
                    # overshoots by one DMA, so make that one cheap
                    nc.sync.dma_start(out=bias_sb, in_=bias[:, :])
            ktd_sb = cpool.tile([128, 5 * 2 * 128], mdt)
            nc.sync.dma_start(out=ktd_sb, in_=ktd[:, :])
            xd = cpool.tile([128, DWIN], mdt)
            nc.sync.dma_start(out=xd, in_=xd4[:, :])
            if bias_pos > len(fchunks):
                nc.sync.dma_start(out=bias_sb, in_=bias[:, :])

            xf_r = xf.rearrange("p (r w) -> p r w", w=WP)
            xd_r = xd.rearrange("p (r w) -> p r w", w=WP)

            pts = [(0, 0), (0, 16), (2, 0), (2, 16)]
            tiles = []
            for k, (dbase, h0) in enumerate(pts):
                if k >= len(pts) - tail_split:
                    tiles.append((dbase, h0, 8))
                    tiles.append((dbase, h0 + 8, 8))
                else:
                    tiles.append((dbase, h0, 16))
            psums = {
                t: ppool.tile([128, 32 * t[2]], f32, name=f"ps_{t[0]}_{t[1]}")
                for t in tiles
            }
            obufs = {
                t: cpool.tile([128, 32 * t[2]], f32, name=f"ob_{t[0]}_{t[1]}")
                for t in tiles
            }

            def main_mms(t):
                dbase, h0, nr = t
                ps = psums[t]
                for m in range(6):
                    for j in range(5):
                        r = (dbase + m) * HP + h0 + j
                        c = (5 - m) * 64
                        nc.tensor.matmul(
                            ps,
                            ktm_sb[:, j * 448 + c : j * 448 + c + 128],
                            xf_r[:, r : r + nr, 0:W],
                            start=(m == 0 and j == 0),
                            stop=False,
                        )

            def close_mms(t):
                dbase, h0, nr = t
                ps = psums[t]
                for pi, p in enumerate((0, 2)):
                    for j in range(5):
                        r = (dbase + p) * HP + h0 + j
                        nc.tensor.matmul(
                            ps,
                            ktd_sb[:, j * 256 + pi * 128 : j * 256 + pi * 128 + 128],
                            xd_r[:, r : r + nr, 0:W],
                            start=False,
                            stop=(pi == 1 and j == 4),
                        )

            def epilogue(t, qi=0):
                dbase, h0, nr = t
                ps = psums[t]
                ob = obufs[t]
                nc.vector.tensor_scalar_add(out=ob[:, :], in0=ps[:, :],
                                            scalar1=bias_sb)
                q = qmap[store_q[qi % len(store_q)]]
                q.dma_start(
                    out=out[dbase : dbase + 2, :, h0 * W : (h0 + nr) * W],
                    in_=ob[:, :],
                )

            half = len(tiles) // 2
            qi = 0
            for grp in (tiles[:half], tiles[half:]):
                for t in grp:
                    main_mms(t)
                for t in grp:
                    close_mms(t)
                    epilogue(t, qi)
                    qi += 1
    nc.finalize()
    _NC_CACHE[key] = nc
    return nc


def _prep_v3_weights(K, bias, mnp):
    """Descending-l block layouts: psum[0:64]=plane dbase, [64:128]=dbase+1."""
    # ktm[(delta,ic), j*448 + b*64 + o] = K[o, ic, 5-b, j, delta], b in 1..5
    ktm = np.zeros((4, IC, 5, 7, OC), np.float32)
    ktm[:, :, :, 1:6, :] = K.transpose(4, 1, 3, 2, 0)[0:4][:, :, :, ::-1, :]
    ktm = ktm.reshape(128, 5 * 7 * OC).astype(mnp)
    ktd = np.zeros((4, IC, 5, 2, 128), np.float32)
    Kd = K[:, :, :, :, 4]  # (oc, ic, l, j)
    for lam in range(4):
        for j in range(5):
            # window A (p=0): T0 cols 0:64 <- l=lam; T1 cols 64:128 <- l=lam-1
            ktd[lam, :, j, 0, 0:64] = Kd[:, :, lam, j].T
            if lam >= 1:
                ktd[lam, :, j, 0, 64:128] = Kd[:, :, lam - 1, j].T
            # window B (p=2): T0 <- l=4 (lam 2); T1 <- l=lam+1 (lam 2,3)
            if lam == 2:
                ktd[lam, :, j, 1, 0:64] = Kd[:, :, 4, j].T
            if lam >= 2:
                ktd[lam, :, j, 1, 64:128] = Kd[:, :, lam + 1, j].T
    ktd = ktd.reshape(128, 5 * 2 * 128).astype(mnp)
    bias128 = np.ascontiguousarray(
        np.tile(bias.reshape(1, OC), (2, 1)).reshape(128, 1)
    )
    return {"ktm": np.ascontiguousarray(ktm), "ktd": np.ascontiguousarray(ktd),
            "bias": bias128}


def _prep_v2_weights(K, bias, mnp):
    """Host-side lhsT layouts for v2 (see _build_nc_v2)."""
    # ktm[(delta,ic), j*448 + b*64 + o] = K[o, ic, b-1, j, delta], b in 1..5
    ktm = np.zeros((4, IC, 5, 7, OC), np.float32)
    ktm[:, :, :, 1:6, :] = K.transpose(4, 1, 3, 2, 0)[0:4]
    ktm = ktm.reshape(128, 5 * 7 * OC).astype(mnp)
    # ktd[(lam,ic), j*256 + v*128 + c]
    ktd = np.zeros((4, IC, 5, 2, 128), np.float32)
    Kd = K[:, :, :, :, 4]  # (oc, ic, l, j)
    for lam in range(4):
        for j in range(5):
            # window A (p=0): T1 cols 0:64 <- l=lam-1; T0 cols 64:128 <- l=lam
            if lam >= 1:
                ktd[lam, :, j, 0, 0:64] = Kd[:, :, lam - 1, j].T
            ktd[lam, :, j, 0, 64:128] = Kd[:, :, lam, j].T
            # window B (p=2): T1 <- l=lam+1 (lam 2,3); T0 <- l=4 (lam 2)
            if lam >= 2:
                ktd[lam, :, j, 1, 0:64] = Kd[:, :, lam + 1, j].T
            if lam == 2:
                ktd[lam, :, j, 1, 64:128] = Kd[:, :, 4, j].T
    ktd = ktd.reshape(128, 5 * 2 * 128).astype(mnp)
    bias128 = np.ascontiguousarray(
        np.tile(bias.reshape(1, OC), (2, 1)).reshape(128, 1)
    )
    return {"ktm": np.ascontiguousarray(ktm), "ktd": np.ascontiguousarray(ktd),
            "bias": bias128}


def _build_nc(mm="bf16"):
    key = ("v0", mm)
    if key in _NC_CACHE:
        return _NC_CACHE[key]
    f32 = mybir.dt.float32
    mdt = {"f32": f32, "bf16": mybir.dt.bfloat16}[mm]
    nc = bacc.Bacc()
    xs = nc.dram_tensor("xs", [IC, XS_COLS], mdt, kind="ExternalInput")
    kt = nc.dram_tensor("kt", [128, NTAPS_LJ * OC], mdt, kind="ExternalInput")
    kt4 = nc.dram_tensor("kt4", [IC, NTAPS_LJ * OC], mdt, kind="ExternalInput")
    bias = nc.dram_tensor("bias", [OC, 1], f32, kind="ExternalInput")
    out = nc.dram_tensor("out", [OC, OUTF], f32, kind="ExternalOutput")

    with TileContext(nc) as tc:
        with (
            tc.tile_pool(name="const", bufs=1) as cpool,
            tc.tile_pool(name="psum", bufs=4, space="PSUM") as ppool,
        ):
            xrep = cpool.tile([128, SLABF], mdt)
            # partition p = dl*32+ic holds xs[ic, dl : dl+SLABF] (w-shift by dl)
            for dl in range(4):
                nc.sync.dma_start(
                    out=xrep[dl * IC : (dl + 1) * IC, :], in_=xs[:, dl : dl + SLABF]
                )
            kt_sb = cpool.tile([128, NTAPS_LJ * OC], mdt)
            nc.sync.dma_start(out=kt_sb, in_=kt[:, :])
            kt4_sb = cpool.tile([IC, NTAPS_LJ * OC], mdt)
            nc.sync.dma_start(out=kt4_sb, in_=kt4[:, :])
            bias_sb = cpool.tile([OC, 1], f32)
            nc.sync.dma_start(out=bias_sb, in_=bias[:, :])
            obuf = cpool.tile([OC, OUTF], f32)

            # view xrep free dim as (row, w) where row = d*HP + h
            xrep_r = xrep.rearrange("p (r w) -> p r w", w=WP)

            for t in range(8):  # out tile: 512 outputs = 16 h-rows x 32 w
                d, h0 = divmod(t, 2)
                h0 *= 16
                ps = ppool.tile([OC, 512], f32)
                for lj in range(NTAPS_LJ):
                    l, j = divmod(lj, 5)
                    r = (d + l) * HP + h0 + j
                    rhs = xrep_r[:, r : r + 16, 0:W]
                    nc.tensor.matmul(
                        ps,
                        kt_sb[:, lj * OC : (lj + 1) * OC],
                        rhs,
                        start=(lj == 0),
                        stop=False,
                    )
                    rhs4 = xrep_r[0:IC, r : r + 16, 4 : 4 + W]
                    nc.tensor.matmul(
                        ps,
                        kt4_sb[:, lj * OC : (lj + 1) * OC],
                        rhs4,
                        start=False,
                        stop=(lj == NTAPS_LJ - 1),
                    )
                nc.vector.tensor_scalar_add(
                    out=obuf[:, t * 512 : (t + 1) * 512], in0=ps, scalar1=bias_sb
                )
            nc.sync.dma_start(out=out[:, :], in_=obuf)
    nc.finalize()
    _NC_CACHE[key] = nc
    return nc


_V3_OPTS = {}


def kernel(x, weight, P, bias, mm="bf16", ver="v3"):
    import ml_dtypes

    global _last_in_maps, _last_mm, _last_build
    x = np.ascontiguousarray(np.asarray(x, dtype=np.float32))
    weight = np.asarray(weight, dtype=np.float32)
    P = np.asarray(P, dtype=np.float32)
    bias = np.asarray(bias, dtype=np.float32)
    mnp = {"f32": np.float32, "bf16": ml_dtypes.bfloat16}[mm]

    K = _construct_K(weight, P)  # (oc, ic, l, j, i)
    # lhsT layouts: partition=(i, ic), free=(l*5+j slot, oc)
    Kt = K.transpose(4, 1, 2, 3, 0)  # (i, ic, l, j, oc)
    KtF = Kt.reshape(5, IC, NTAPS_LJ, OC)
    bias_in = np.ascontiguousarray(bias.reshape(OC, 1))

    xpad = np.pad(x, ((0, 0), (0, 0), (PAD, PAD), (PAD, PAD), (PAD, PAD)))

    if ver == "v3":
        PL = HP * WP
        extra = _prep_v3_weights(K, bias, mnp)
        in_maps = []
        for ci in range(8):
            n, dc = divmod(ci, 4)
            slab = xpad[n, :, 4 * dc : 4 * dc + DSLAB].reshape(IC, SLABF)
            xsf = np.zeros((IC, XS_COLS), np.float32)
            xsf[:, :SLABF] = slab
            xs4 = np.stack(
                [xsf[:, d : d + 8 * PL] for d in range(4)]
            ).reshape(128, 8 * PL).astype(mnp)
            xd4 = np.stack(
                [xsf[:, l * PL + 4 : l * PL + 4 + 5 * PL] for l in range(4)]
            ).reshape(128, 5 * PL).astype(mnp)
            in_maps.append({"xs4": xs4, "xd4": xd4, **extra})
        _last_in_maps = in_maps
        _last_mm = mm
        _last_build = lambda m: _build_nc_v3(m, **_V3_OPTS)
        nc = _last_build(mm)
        res = run_bass_kernel_spmd(nc, in_maps, core_ids=list(range(8)))
        out = np.empty((N, OC, D, H, W), np.float32)
        for ci in range(8):
            n, dc = divmod(ci, 4)
            o4 = res.results[ci]["out"]  # (4, 64, 1024)
            out[n, :, 4 * dc : 4 * dc + DCHUNK] = o4.transpose(1, 0, 2).reshape(
                OC, DCHUNK, H, W
            )
        return out

    if ver == "v2":
        extra = _prep_v2_weights(K, bias, mnp)
        build = _build_nc_v2
        in_maps = []
        for ci in range(8):
            n, dc = divmod(ci, 4)
            slab = xpad[n, :, 4 * dc : 4 * dc + DSLAB].reshape(IC, SLABF)
            xs = np.zeros((IC, XS_COLS), mnp)
            xs[:, :SLABF] = slab.astype(mnp)
            in_maps.append({"xs": xs, **extra})
        _last_in_maps = in_maps
        _last_mm = mm
        _last_build = build
        nc = build(mm)
        res = run_bass_kernel_spmd(nc, in_maps, core_ids=list(range(8)))
        out = np.empty((N, OC, D, H, W), np.float32)
        for ci in range(8):
            n, dc = divmod(ci, 4)
            out[n, :, 4 * dc : 4 * dc + DCHUNK] = res.results[ci]["out"].reshape(
                OC, DCHUNK, H, W
            )
        return out

    if ver == "v0":
        kt = np.ascontiguousarray(KtF[:4].reshape(128, NTAPS_LJ * OC).astype(mnp))
        kt4 = np.ascontiguousarray(KtF[4].reshape(IC, NTAPS_LJ * OC).astype(mnp))
        extra = {"kt": kt, "kt4": kt4}
        build = _build_nc
    else:
        kta = np.ascontiguousarray(
            KtF[:4][:, :, LJ_A, :].reshape(128, len(LJ_A) * OC).astype(mnp)
        )
        ktb = np.ascontiguousarray(
            KtF[:4][:, :, LJ_B, :].reshape(128, len(LJ_B) * OC).astype(mnp)
        )
        # ktd: partition (l, ic) for l=0..3, free (j, oc): taps (l, j, i=4)
        ktd = np.zeros((128, 5 * OC), mnp)
        for j in range(5):
            for l in range(4):
                ktd[32 * l : 32 * (l + 1), j * OC : (j + 1) * OC] = KtF[
                    4, :, l * 5 + j, :
                ].astype(mnp)
        # ktj: partition (j, ic) for j=0..3: taps (l=4, j, i=4)
        ktj = np.zeros((128, OC), mnp)
        for j in range(4):
            ktj[32 * j : 32 * (j + 1), :] = KtF[4, :, 4 * 5 + j, :].astype(mnp)
        kt5 = np.ascontiguousarray(KtF[4, :, 24, :].astype(mnp))  # (l=4,j=4,i=4)
        extra = {"kta": kta, "ktb": ktb, "ktd": ktd, "ktj": ktj, "kt5": kt5}
        build = _build_nc_packed

    in_maps = []
    for ci in range(8):
        n, dc = divmod(ci, 4)
        slab = xpad[n, :, 4 * dc : 4 * dc + DSLAB].reshape(IC, SLABF)
        xs = np.zeros((IC, XS_COLS), mnp)
        xs[:, :SLABF] = slab.astype(mnp)
        in_maps.append({"xs": xs, "bias": bias_in, **extra})

    _last_in_maps = in_maps
    _last_mm = mm
    _last_build = build
    nc = build(mm)
    res = run_bass_kernel_spmd(nc, in_maps, core_ids=list(range(8)))

    out = np.empty((N, OC, D, H, W), np.float32)
    for ci in range(8):
        n, dc = divmod(ci, 4)
        out[n, :, 4 * dc : 4 * dc + DCHUNK] = res.results[ci]["out"].reshape(
            OC, DCHUNK, H, W
        )
    return out



# revision 27
# speedup vs baseline: 2.0784x; 1.0143x over previous
"""Dcls3d (learnable-position dilated conv3d) Trainium2 kernel.

Reference computes:
  K = trilinear-scatter(weight, P) -> (64, 32, 5, 5, 5)
  out = conv3d(x, K, stride 1, pad 2) + bias     x: (2,32,16,32,32) -> out: (2,64,16,32,32)

Strategy (8 cores): shard (batch n in {0,1}) x (4 chunks of 4 output d-planes).
Each core runs an implicit-GEMM direct conv:
  - input slab (zero-padded on host) replicated 4x in SBUF, w-shifted by
    delta=0..3, giving a 128-partition (delta, ic) contraction axis.
  - for each of 25 (l, j) kernel-tap pairs: one matmul contracting
    (4 w-taps x 32 ic) = 128, M=64 out-channels, N=512 outputs, accumulating
    in PSUM; the i=4 leftover tap runs as a K=32 matmul off the delta-group.
  - bias added during PSUM->SBUF copyback; one 1MB store per core.
"""

import numpy as np

import concourse.bass as bass
import concourse.bacc as bacc
import concourse.mybir as mybir
from concourse.bass_utils import run_bass_kernel_spmd
from concourse.tile import TileContext

# ---- problem constants (hardcoded per contract) ----
N, IC, D, H, W = 2, 32, 16, 32, 32
OC = 64
KC = 16
PAD = 2
DP, HP, WP = D + 2 * PAD, H + 2 * PAD, W + 2 * PAD  # 20, 36, 36
DCHUNK = 4              # output d-planes per core
DSLAB = DCHUNK + 4      # input d-planes per core (halo 2 each side)
SLABF = DSLAB * HP * WP  # 8*36*36 = 10368
XS_COLS = SLABF + 4     # slack so the delta-shifted loads stay in bounds
NTAPS_LJ = 25
OUTF = DCHUNK * H * W   # 4096 outputs per (core, oc)

_NC_CACHE = {}


def _construct_K(weight, P):
    """Exact numpy port of reference.construct_kernel for ks=(5,5,5)."""
    Pp = P + np.float32(2.0)
    Pf = np.floor(Pp)
    R = Pp - Pf
    P1, P2, P3 = Pf[0], Pf[1], Pf[2]
    R1, R2, R3 = R[0], R[1], R[2]
    g = np.arange(5, dtype=P.dtype)[:, None, None, None]
    aL = (g == P1) * (1.0 - R1) + (g == P1 + 1.0) * R1
    aJ = (g == P3) * (1.0 - R3) + (g == P3 + 1.0) * R3
    aI = (g == P2) * (1.0 - R2) + (g == P2 + 1.0) * R2
    K = np.einsum("ock,lock,jock,iock->oclji", weight, aL, aJ, aI, optimize=True)
    return np.ascontiguousarray(K.astype(np.float32))


LJ_A = [lj for lj in range(NTAPS_LJ) if lj % 2 == 0]  # col-group 0 taps
LJ_B = [lj for lj in range(NTAPS_LJ) if lj % 2 == 1]  # col-group 1 taps
ROW_PACK = False  # leftover i=4 taps spread across PE row groups


def _build_nc_packed(mm="bf16"):
    """v1: col-group packed (2 taps concurrently on PE) + row-packed i=4."""
    key = ("v1", mm, ROW_PACK)
    if key in _NC_CACHE:
        return _NC_CACHE[key]
    f32 = mybir.dt.float32
    mdt = {"f32": f32, "bf16": mybir.dt.bfloat16}[mm]
    nc = bacc.Bacc()
    xs = nc.dram_tensor("xs", [IC, XS_COLS], mdt, kind="ExternalInput")
    kta = nc.dram_tensor("kta", [128, len(LJ_A) * OC], mdt, kind="ExternalInput")
    ktb = nc.dram_tensor("ktb", [128, len(LJ_B) * OC], mdt, kind="ExternalInput")
    ktd = nc.dram_tensor("ktd", [128, 5 * OC], mdt, kind="ExternalInput")
    ktj = nc.dram_tensor("ktj", [128, OC], mdt, kind="ExternalInput")
    kt5 = nc.dram_tensor("kt5", [IC, OC], mdt, kind="ExternalInput")
    bias = nc.dram_tensor("bias", [OC, 1], f32, kind="ExternalInput")
    out = nc.dram_tensor("out", [OC, OUTF], f32, kind="ExternalOutput")

    HALF = 6 * HP * WP  # six d-planes per xrep half
    with TileContext(nc) as tc:
        with (
            tc.tile_pool(name="const", bufs=1) as cpool,
            tc.tile_pool(name="psum", bufs=8, space="PSUM") as ppool,
        ):
            kta_sb = cpool.tile([128, len(LJ_A) * OC], mdt)
            nc.sync.dma_start(out=kta_sb, in_=kta[:, :])
            ktb_sb = cpool.tile([128, len(LJ_B) * OC], mdt)
            nc.sync.dma_start(out=ktb_sb, in_=ktb[:, :])
            ktd_sb = cpool.tile([128, 5 * OC], mdt)
            nc.sync.dma_start(out=ktd_sb, in_=ktd[:, :])
            ktj_sb = cpool.tile([128, OC], mdt)
            nc.sync.dma_start(out=ktj_sb, in_=ktj[:, :])
            kt5_sb = cpool.tile([IC, OC], mdt)
            nc.sync.dma_start(out=kt5_sb, in_=kt5[:, :])
            bias_sb = cpool.tile([OC, 1], f32)
            nc.sync.dma_start(out=bias_sb, in_=bias[:, :])
            # input slab split in two halves (planes 0-5 / 2-7) so out d=0,1
            # compute starts while the second half still loads
            xrepA = cpool.tile([128, HALF], mdt)
            xrepB = cpool.tile([128, HALF], mdt)
            for dl in range(4):
                nc.sync.dma_start(
                    out=xrepA[dl * IC : (dl + 1) * IC, :], in_=xs[:, dl : dl + HALF]
                )
            for dl in range(4):
                nc.sync.dma_start(
                    out=xrepB[dl * IC : (dl + 1) * IC, :],
                    in_=xs[:, 2 * HP * WP + dl : 2 * HP * WP + dl + HALF],
                )
            obufs = [cpool.tile([OC, H * W], f32, name=f"obuf{d}") for d in range(4)]

            # d-shifted replication for the i=4 taps: partition group
            # lam holds xs shifted by lam d-planes AND +4 in w, so one
            # K=128 matmul covers taps (l=lam, j, i=4) for lam=0..3.
            DWIN = 4 * HP * WP
            xrepD = cpool.tile([128, DWIN], mdt)
            for lam in range(4):
                o = lam * HP * WP + 4
                nc.sync.dma_start(
                    out=xrepD[lam * IC : (lam + 1) * IC, :], in_=xs[:, o : o + DWIN]
                )
            # h-row (j) shifted replication for taps (l=4, j=0..3, i=4):
            # partition group mu holds planes 4..7 shifted by mu rows and +4 w
            JWIN = 5040
            xrepJ = cpool.tile([128, JWIN], mdt)
            for mu in range(4):
                o = 4 * HP * WP + mu * WP + 4
                nc.sync.dma_start(
                    out=xrepJ[mu * IC : (mu + 1) * IC, :], in_=xs[:, o : o + JWIN]
                )

            xrepA_r = xrepA.rearrange("p (r w) -> p r w", w=WP)
            xrepB_r = xrepB.rearrange("p (r w) -> p r w", w=WP)
            xrepD_r = xrepD.rearrange("p (r w) -> p r w", w=WP)
            xrepJ_r = xrepJ.rearrange("p (r w) -> p r w", w=WP)

            def tile_geom(t):
                d, h0 = divmod(t, 2)
                h0 *= 16
                xr = xrepA_r if d < 2 else xrepB_r
                dbase = 0 if d < 2 else 2
                return d, h0, xr, dbase

            # pass 1: all w-packed taps (need only xrepA/xrepB) for all 8
            # tiles -- 8 psum banks accumulate concurrently, so the PE never
            # stalls on the later xrepD/xrepJ DMAs.
            pss = []
            for t in range(8):
                d, h0, xrep_r, dbase = tile_geom(t)
                ps = ppool.tile([128, 512], f32)
                pss.append(ps)
                for s in range(len(LJ_A)):
                    for grp, ljs, ktsb in ((0, LJ_A, kta_sb), (1, LJ_B, ktb_sb)):
                        if s >= len(ljs):
                            continue
                        lj = ljs[s]
                        l, j = divmod(lj, 5)
                        r = (d + l - dbase) * HP + h0 + j
                        nc.tensor.matmul(
                            ps[grp * 64 : grp * 64 + 64, :],
                            ktsb[:, s * OC : (s + 1) * OC],
                            xrep_r[:, r : r + 16, 0:W],
                            start=(s == 0),
                            stop=False,
                            skip_group_check=True,
                            tile_position=(0, grp * 64),
                        )
            # pass 2: i=4 closers off xrepD/xrepJ + corner single + epilogue
            for t in range(8):
                d, h0, xrep_r, dbase = tile_geom(t)
                ps = pss[t]
                for j in range(5):
                    grp = j % 2
                    nc.tensor.matmul(
                        ps[grp * 64 : grp * 64 + 64, :],
                        ktd_sb[:, j * OC : (j + 1) * OC],
                        xrepD_r[:, d * HP + h0 + j : d * HP + h0 + j + 16, 0:W],
                        start=False,
                        stop=False,
                        skip_group_check=True,
                        tile_position=(0, grp * 64),
                    )
                nc.tensor.matmul(
                    ps[64:128, :],
                    ktj_sb[:, :],
                    xrepJ_r[:, d * HP + h0 : d * HP + h0 + 16, 0:W],
                    start=False,
                    stop=True,
                    skip_group_check=True,
                    tile_position=(0, 64),
                )
                r45 = (d + 4 - dbase) * HP + h0 + 4  # tap (l=4, j=4)
                nc.tensor.matmul(
                    ps[0:64, :],
                    kt5_sb[0:IC, :],
                    xrep_r[0:IC, r45 : r45 + 16, 4 : 4 + W],
                    start=False,
                    stop=True,
                    skip_group_check=True,
                    tile_position=(0, 0),
                )
                oslice = obufs[d][:, (t % 2) * 512 : (t % 2) * 512 + 512]
                nc.vector.tensor_scalar_add(out=oslice, in0=ps[0:64, :], scalar1=bias_sb)
                nc.vector.tensor_tensor(
                    out=oslice, in0=ps[64:128, :], in1=oslice,
                    op=mybir.AluOpType.add,
                )
                if t % 2 == 1:
                    nc.sync.dma_start(
                        out=out[:, d * H * W : (d + 1) * H * W], in_=obufs[d]
                    )
    nc.finalize()
    _NC_CACHE[key] = nc
    return nc


def _build_nc_v2(mm="bf16", warm=9, fchunks=((0, 2), (2, 5), (5, 8)),
                 bias_late=False, tail_split=0):
    """v2: plane-paired M=128 matmuls via sliding-window weight layout.

    Pair-tile = (dbase in {0,2}, h0 in {0,16}): psum[0:64] = out plane
    dbase+1, psum[64:128] = plane dbase, N = 512 (16 h-rows x 32 w).
    Main taps (i=0..3): windows (m in 0..5, j in 0..4); lhsT slides over a
    7-block [Z|l0..l4|Z] column layout so one K=128 matmul feeds both
    planes. i=4 taps: 2 windows x 5 j on a d-shifted (+4w) stack.
    152 matmuls total vs 256 in v1.
    """
    key = ("v2", mm, warm, fchunks, bias_late, tail_split)
    if key in _NC_CACHE:
        return _NC_CACHE[key]
    f32 = mybir.dt.float32
    mdt = {"f32": f32, "bf16": mybir.dt.bfloat16}[mm]
    nc = bacc.Bacc()
    xs = nc.dram_tensor("xs", [IC, XS_COLS], mdt, kind="ExternalInput")
    ktm = nc.dram_tensor("ktm", [128, 5 * 7 * OC], mdt, kind="ExternalInput")
    ktd = nc.dram_tensor("ktd", [128, 5 * 2 * 128], mdt, kind="ExternalInput")
    bias = nc.dram_tensor("bias", [128, 1], f32, kind="ExternalInput")
    out = nc.dram_tensor("out", [OC, OUTF], f32, kind="ExternalOutput")

    PL = HP * WP  # 1296 elems per padded plane
    DWIN = 5 * PL  # xd free extent (view planes 0..4 -> x planes lam..lam+4)
    with TileContext(nc) as tc:
        with (
            tc.tile_pool(name="const", bufs=1) as cpool,
            tc.tile_pool(name="psum", bufs=1, space="PSUM") as ppool,
        ):
            # PE warmup: ramp the clock to full p-state during the input DMAs
            wsb = cpool.tile([128, 512], mdt)
            nc.gpsimd.memset(wsb[:, :], 0.0)
            wps = ppool.tile([128, 512], f32)
            for _ in range(warm):
                nc.tensor.matmul(wps, wsb[:, 0:128], wsb[:, 0:512],
                                 start=True, stop=True)

            bias_sb = cpool.tile([128, 1], f32)
            if not bias_late:
                nc.sync.dma_start(out=bias_sb, in_=bias[:, :])
            ktm_sb = cpool.tile([128, 5 * 7 * OC], mdt)
            nc.sync.dma_start(out=ktm_sb, in_=ktm[:, :])
            # w-shift stack: partition (delta, ic) holds xs[ic, c+delta]
            xf = cpool.tile([128, 8 * PL], mdt)
            for p0, p1 in fchunks[:2]:
                for dl in range(4):
                    nc.sync.dma_start(
                        out=xf[dl * IC : (dl + 1) * IC, p0 * PL : p1 * PL],
                        in_=xs[:, p0 * PL + dl : p1 * PL + dl],
                    )
            if bias_late:
                nc.sync.dma_start(out=bias_sb, in_=bias[:, :])
            ktd_sb = cpool.tile([128, 5 * 2 * 128], mdt)
            nc.sync.dma_start(out=ktd_sb, in_=ktd[:, :])
            # d-shift stack for i=4: partition (lam, ic) = xs[ic, c+lam*PL+4]
            xd = cpool.tile([128, DWIN], mdt)
            for lam in range(4):
                nc.sync.dma_start(
                    out=xd[lam * IC : (lam + 1) * IC, :],
                    in_=xs[:, lam * PL + 4 : lam * PL + 4 + DWIN],
                )
            for p0, p1 in fchunks[2:]:
                for dl in range(4):
                    nc.sync.dma_start(
                        out=xf[dl * IC : (dl + 1) * IC, p0 * PL : p1 * PL],
                        in_=xs[:, p0 * PL + dl : p1 * PL + dl],
                    )

            xf_r = xf.rearrange("p (r w) -> p r w", w=WP)
            xd_r = xd.rearrange("p (r w) -> p r w", w=WP)

            # tile descriptors: (dbase, h0, nrows); the last `tail_split`
            # pair-tiles are split into two half-width psum groups so the
            # first half's epilogue+stores overlap the second half's matmuls
            pts = [(0, 0), (0, 16), (2, 0), (2, 16)]
            tiles = []
            for k, (dbase, h0) in enumerate(pts):
                if k >= len(pts) - tail_split:
                    tiles.append((dbase, h0, 8))
                    tiles.append((dbase, h0 + 8, 8))
                else:
                    tiles.append((dbase, h0, 16))
            psums = {
                t: ppool.tile([128, 32 * t[2]], f32, name=f"ps_{t[0]}_{t[1]}")
                for t in tiles
            }
            obufs = {
                t: cpool.tile([128, 32 * t[2]], f32, name=f"ob_{t[0]}_{t[1]}")
                for t in tiles
            }

            def main_mms(t):
                dbase, h0, nr = t
                ps = psums[t]
                for m in range(6):
                    for j in range(5):
                        r = (dbase + m) * HP + h0 + j
                        nc.tensor.matmul(
                            ps,
                            ktm_sb[:, j * 448 + m * 64 : j * 448 + m * 64 + 128],
                            xf_r[:, r : r + nr, 0:W],
                            start=(m == 0 and j == 0),
                            stop=False,
                        )

            def close_mms(t):
                dbase, h0, nr = t
                ps = psums[t]
                for pi, p in enumerate((0, 2)):
                    for j in range(5):
                        r = (dbase + p) * HP + h0 + j
                        nc.tensor.matmul(
                            ps,
                            ktd_sb[:, j * 256 + pi * 128 : j * 256 + pi * 128 + 128],
                            xd_r[:, r : r + nr, 0:W],
                            start=False,
                            stop=(pi == 1 and j == 4),
                        )

            def epilogue(t, q0=None, q1=None):
                dbase, h0, nr = t
                ps = psums[t]
                ob = obufs[t]
                nc.vector.tensor_scalar_add(out=ob[:, :], in0=ps[:, :],
                                            scalar1=bias_sb)
                base = dbase * H * W + h0 * W
                (q0 or nc.scalar).dma_start(out=out[:, base : base + nr * W],
                                            in_=ob[64:128, :])
                base1 = (dbase + 1) * H * W + h0 * W
                (q1 or nc.scalar).dma_start(out=out[:, base1 : base1 + nr * W],
                                            in_=ob[0:64, :])

            # compute order: mains of a dbase-pair back to back, then closers
            # (xd loads later than xf), epilogues as each psum completes
            half = len(tiles) // 2
            for gi, grp in enumerate((tiles[:half], tiles[half:])):
                for t in grp:
                    main_mms(t)
                for k, t in enumerate(grp):
                    close_mms(t)
                    last = gi == 1 and k >= len(grp) - 2
                    epilogue(t, q0=nc.sync if last else None,
                             q1=nc.scalar if last else None)
    nc.finalize()
    _NC_CACHE[key] = nc
    return nc


def _build_nc_v3(mm="bf16", warm=8, fchunks=(1, 1, 2, 2, 2), tail_split=2,
                 store_q=("scalar", "sync"), bias_pos=1, first_cols=756,
                 last_rows=(8, 8)):
    """v3: like v2 but shift-replication is host-side (xs4/xd4 in DRAM), so
    each chunk load is ONE DMA; weight blocks in descending-l order so psum
    halves map to ascending planes, enabling one 3D-AP store per pair-tile.
    """
    key = ("v3", mm, warm, fchunks, tail_split, store_q, bias_pos, first_cols,
           last_rows)
    if key in _NC_CACHE:
        return _NC_CACHE[key]
    f32 = mybir.dt.float32
    mdt = {"f32": f32, "bf16": mybir.dt.bfloat16}[mm]
    nc = bacc.Bacc()
    PL = HP * WP  # 1296
    DWIN = 5 * PL
    xs4 = nc.dram_tensor("xs4", [128, 8 * PL], mdt, kind="ExternalInput")
    xd4 = nc.dram_tensor("xd4", [128, DWIN], mdt, kind="ExternalInput")
    ktm = nc.dram_tensor("ktm", [128, 31 * OC], mdt, kind="ExternalInput")
    ktd = nc.dram_tensor("ktd", [128, 5 * 2 * 128], mdt, kind="ExternalInput")
    bias = nc.dram_tensor("bias", [128, 1], f32, kind="ExternalInput")
    out = nc.dram_tensor("out", [DCHUNK, OC, H * W], f32, kind="ExternalOutput")

    qmap = {"scalar": None, "sync": None, "vector": None, "gpsimd": None}
    with TileContext(nc) as tc:
        with (
            tc.tile_pool(name="const", bufs=1) as cpool,
            tc.tile_pool(name="psum", bufs=1, space="PSUM") as ppool,
        ):
            qmap = {"scalar": nc.scalar, "sync": nc.sync, "vector": nc.vector,
                    "gpsimd": nc.gpsimd}
            wsb = cpool.tile([128, 512], mdt)
            nc.gpsimd.memset(wsb[:, :], 0.0)
            wps = ppool.tile([128, 512], f32)
            for _ in range(warm):
                nc.tensor.matmul(wps, wsb[:, 0:128], wsb[:, 0:512],
                                 start=True, stop=True)

            ktm_sb = cpool.tile([128, 31 * OC], mdt)
            nc.sync.dma_start(out=ktm_sb, in_=ktm[:, :])
            bias_sb = cpool.tile([128, 1], f32)
            xf = cpool.tile([128, 8 * PL], mdt)
            bounds = [0]
            if first_cols:
                bounds.append(first_cols)
            p0 = 0
            for span in fchunks:
                p0 += span
                bounds.append(p0 * PL)
            for k in range(len(bounds) - 1):
                c0, c1 = bounds[k], bounds[k + 1]
                nc.sync.dma_start(out=xf[:, c0:c1], in_=xs4[:, c0:c1])
                if k + 1 == bias_pos:
                    # tiny DMA placed here: the tile scheduler's sem target
                    # overshoots by one DMA, so make that one cheap
                    nc.sync.dma_start(out=bias_sb, in_=bias[:, :])
            ktd_sb = cpool.tile([128, 5 * 2 * 128], mdt)
            nc.sync.dma_start(out=ktd_sb, in_=ktd[:, :])
            xd = cpool.tile([128, DWIN], mdt)
            nc.sync.dma_start(out=xd, in_=xd4[:, :])
            if bias_pos > len(fchunks):
                nc.sync.dma_start(out=bias_sb, in_=bias[:, :])

            xf_r = xf.rearrange("p (r w) -> p r w", w=WP)
            xd_r = xd.rearrange("p (r w) -> p r w", w=WP)

            pts = [(0, 0), (0, 16), (2, 0), (2, 16)]
            tiles = []
            for k, (dbase, h0) in enumerate(pts):
                if k >= len(pts) - tail_split:
                    rows = last_rows if k == len(pts) - 1 else (8, 8)
                    hh = h0
                    for nr in rows:
                        tiles.append((dbase, hh, nr))
                        hh += nr
                else:
                    tiles.append((dbase, h0, 16))
            psums = {
                t: ppool.tile([128, 32 * t[2]], f32, name=f"ps_{t[0]}_{t[1]}")
                for t in tiles
            }
            obufs = {
                t: cpool.tile([128, 32 * t[2]], f32, name=f"ob_{t[0]}_{t[1]}")
                for t in tiles
            }

            def main_mms(t):
                dbase, h0, nr = t
                ps = psums[t]
                for m in range(6):
                    for j in range(5):
                        r = (dbase + m) * HP + h0 + j
                        c = (5 + 6 * j - m) * 64
                        nc.tensor.matmul(
                            ps,
                            ktm_sb[:, c : c + 128],
                            xf_r[:, r : r + nr, 0:W],
                            start=(m == 0 and j == 0),
                            stop=False,
                        )

            def close_mms(t):
                dbase, h0, nr = t
                ps = psums[t]
                for pi, p in enumerate((0, 2)):
                    for j in range(5):
                        r = (dbase + p) * HP + h0 + j
                        nc.tensor.matmul(
                            ps,
                            ktd_sb[:, j * 256 + pi * 128 : j * 256 + pi * 128 + 128],
                            xd_r[:, r : r + nr, 0:W],
                            start=False,
                            stop=(pi == 1 and j == 4),
                        )

            def epilogue(t, qi=0):
                dbase, h0, nr = t
                ps = psums[t]
                ob = obufs[t]
                nc.vector.tensor_scalar_add(out=ob[:, :], in0=ps[:, :],
                                            scalar1=bias_sb)
                q = qmap[store_q[qi % len(store_q)]]
                q.dma_start(
                    out=out[dbase : dbase + 2, :, h0 * W : (h0 + nr) * W],
                    in_=ob[:, :],
                )

            half = len(tiles) // 2
            qi = 0
            for grp in (tiles[:half], tiles[half:]):
                for t in grp:
                    main_mms(t)
                for t in grp:
                    close_mms(t)
                    epilogue(t, qi)
                    qi += 1
    nc.finalize()
    _NC_CACHE[key] = nc
    return nc


def _prep_v3_weights(K, bias, mnp):
    """Descending-l blocks with shared zero-blocks between j-groups:
    layout = Z + 5x(l4 l3 l2 l1 l0 Z); block of (j,l) at 1+6j+(4-l).
    psum[0:64]=plane dbase, [64:128]=dbase+1."""
    ktm = np.zeros((4, IC, 31, OC), np.float32)
    kt = K.transpose(4, 1, 3, 2, 0)[0:4]  # (delta, ic, j, l, o)
    for j in range(5):
        ktm[:, :, 1 + 6 * j : 6 + 6 * j, :] = kt[:, :, j, ::-1, :]
    ktm = ktm.reshape(128, 31 * OC).astype(mnp)
    ktd = np.zeros((4, IC, 5, 2, 128), np.float32)
    Kd = K[:, :, :, :, 4]  # (oc, ic, l, j)
    for lam in range(4):
        for j in range(5):
            # window A (p=0): T0 cols 0:64 <- l=lam; T1 cols 64:128 <- l=lam-1
            ktd[lam, :, j, 0, 0:64] = Kd[:, :, lam, j].T
            if lam >= 1:
                ktd[lam, :, j, 0, 64:128] = Kd[:, :, lam - 1, j].T
            # window B (p=2): T0 <- l=4 (lam 2); T1 <- l=lam+1 (lam 2,3)
            if lam == 2:
                ktd[lam, :, j, 1, 0:64] = Kd[:, :, 4, j].T
            if lam >= 2:
                ktd[lam, :, j, 1, 64:128] = Kd[:, :, lam + 1, j].T
    ktd = ktd.reshape(128, 5 * 2 * 128).astype(mnp)
    bias128 = np.ascontiguousarray(
        np.tile(bias.reshape(1, OC), (2, 1)).reshape(128, 1)
    )
    return {"ktm": np.ascontiguousarray(ktm), "ktd": np.ascontiguousarray(ktd),
            "bias": bias128}


def _prep_v2_weights(K, bias, mnp):
    """Host-side lhsT layouts for v2 (see _build_nc_v2)."""
    # ktm[(delta,ic), j*448 + b*64 + o] = K[o, ic, b-1, j, delta], b in 1..5
    ktm = np.zeros((4, IC, 5, 7, OC), np.float32)
    ktm[:, :, :, 1:6, :] = K.transpose(4, 1, 3, 2, 0)[0:4]
    ktm = ktm.reshape(128, 5 * 7 * OC).astype(mnp)
    # ktd[(lam,ic), j*256 + v*128 + c]
    ktd = np.zeros((4, IC, 5, 2, 128), np.float32)
    Kd = K[:, :, :, :, 4]  # (oc, ic, l, j)
    for lam in range(4):
        for j in range(5):
            # window A (p=0): T1 cols 0:64 <- l=lam-1; T0 cols 64:128 <- l=lam
            if lam >= 1:
                ktd[lam, :, j, 0, 0:64] = Kd[:, :, lam - 1, j].T
            ktd[lam, :, j, 0, 64:128] = Kd[:, :, lam, j].T
            # window B (p=2): T1 <- l=lam+1 (lam 2,3); T0 <- l=4 (lam 2)
            if lam >= 2:
                ktd[lam, :, j, 1, 0:64] = Kd[:, :, lam + 1, j].T
            if lam == 2:
                ktd[lam, :, j, 1, 64:128] = Kd[:, :, 4, j].T
    ktd = ktd.reshape(128, 5 * 2 * 128).astype(mnp)
    bias128 = np.ascontiguousarray(
        np.tile(bias.reshape(1, OC), (2, 1)).reshape(128, 1)
    )
    return {"ktm": np.ascontiguousarray(ktm), "ktd": np.ascontiguousarray(ktd),
            "bias": bias128}


def _build_nc(mm="bf16"):
    key = ("v0", mm)
    if key in _NC_CACHE:
        return _NC_CACHE[key]
    f32 = mybir.dt.float32
    mdt = {"f32": f32, "bf16": mybir.dt.bfloat16}[mm]
    nc = bacc.Bacc()
    xs = nc.dram_tensor("xs", [IC, XS_COLS], mdt, kind="ExternalInput")
    kt = nc.dram_tensor("kt", [128, NTAPS_LJ * OC], mdt, kind="ExternalInput")
    kt4 = nc.dram_tensor("kt4", [IC, NTAPS_LJ * OC], mdt, kind="ExternalInput")
    bias = nc.dram_tensor("bias", [OC, 1], f32, kind="ExternalInput")
    out = nc.dram_tensor("out", [OC, OUTF], f32, kind="ExternalOutput")

    with TileContext(nc) as tc:
        with (
            tc.tile_pool(name="const", bufs=1) as cpool,
            tc.tile_pool(name="psum", bufs=4, space="PSUM") as ppool,
        ):
            xrep = cpool.tile([128, SLABF], mdt)
            # partition p = dl*32+ic holds xs[ic, dl : dl+SLABF] (w-shift by dl)
            for dl in range(4):
                nc.sync.dma_start(
                    out=xrep[dl * IC : (dl + 1) * IC, :], in_=xs[:, dl : dl + SLABF]
                )
            kt_sb = cpool.tile([128, NTAPS_LJ * OC], mdt)
            nc.sync.dma_start(out=kt_sb, in_=kt[:, :])
            kt4_sb = cpool.tile([IC, NTAPS_LJ * OC], mdt)
            nc.sync.dma_start(out=kt4_sb, in_=kt4[:, :])
            bias_sb = cpool.tile([OC, 1], f32)
            nc.sync.dma_start(out=bias_sb, in_=bias[:, :])
            obuf = cpool.tile([OC, OUTF], f32)

            # view xrep free dim as (row, w) where row = d*HP + h
            xrep_r = xrep.rearrange("p (r w) -> p r w", w=WP)

            for t in range(8):  # out tile: 512 outputs = 16 h-rows x 32 w
                d, h0 = divmod(t, 2)
                h0 *= 16
                ps = ppool.tile([OC, 512], f32)
                for lj in range(NTAPS_LJ):
                    l, j = divmod(lj, 5)
                    r = (d + l) * HP + h0 + j
                    rhs = xrep_r[:, r : r + 16, 0:W]
                    nc.tensor.matmul(
                        ps,
                        kt_sb[:, lj * OC : (lj + 1) * OC],
                        rhs,
                        start=(lj == 0),
                        stop=False,
                    )
                    rhs4 = xrep_r[0:IC, r : r + 16, 4 : 4 + W]
                    nc.tensor.matmul(
                        ps,
                        kt4_sb[:, lj * OC : (lj + 1) * OC],
                        rhs4,
                        start=False,
                        stop=(lj == NTAPS_LJ - 1),
                    )
                nc.vector.tensor_scalar_add(
                    out=obuf[:, t * 512 : (t + 1) * 512], in0=ps, scalar1=bias_sb
                )
            nc.sync.dma_start(out=out[:, :], in_=obuf)
    nc.finalize()
    _NC_CACHE[key] = nc
    return nc


_V3_OPTS = {}


def kernel(x, weight, P, bias, mm="bf16", ver="v3"):
    import ml_dtypes

    global _last_in_maps, _last_mm, _last_build
    x = np.ascontiguousarray(np.asarray(x, dtype=np.float32))
    weight = np.asarray(weight, dtype=np.float32)
    P = np.asarray(P, dtype=np.float32)
    bias = np.asarray(bias, dtype=np.float32)
    mnp = {"f32": np.float32, "bf16": ml_dtypes.bfloat16}[mm]

    K = _construct_K(weight, P)  # (oc, ic, l, j, i)
    # lhsT layouts: partition=(i, ic), free=(l*5+j slot, oc)
    Kt = K.transpose(4, 1, 2, 3, 0)  # (i, ic, l, j, oc)
    KtF = Kt.reshape(5, IC, NTAPS_LJ, OC)
    bias_in = np.ascontiguousarray(bias.reshape(OC, 1))

    xpad = np.pad(x, ((0, 0), (0, 0), (PAD, PAD), (PAD, PAD), (PAD, PAD)))

    if ver == "v3":
        PL = HP * WP
        extra = _prep_v3_weights(K, bias, mnp)
        in_maps = []
        for ci in range(8):
            n, dc = divmod(ci, 4)
            slab = xpad[n, :, 4 * dc : 4 * dc + DSLAB].reshape(IC, SLABF)
            xsf = np.zeros((IC, XS_COLS), np.float32)
            xsf[:, :SLABF] = slab
            xs4 = np.stack(
                [xsf[:, d : d + 8 * PL] for d in range(4)]
            ).reshape(128, 8 * PL).astype(mnp)
            xd4 = np.stack(
                [xsf[:, l * PL + 4 : l * PL + 4 + 5 * PL] for l in range(4)]
            ).reshape(128, 5 * PL).astype(mnp)
            in_maps.append({"xs4": xs4, "xd4": xd4, **extra})
        _last_in_maps = in_maps
        _last_mm = mm
        _last_build = lambda m: _build_nc_v3(m, **_V3_OPTS)
        nc = _last_build(mm)
        res = run_bass_kernel_spmd(nc, in_maps, core_ids=list(range(8)))
        out = np.empty((N, OC, D, H, W), np.float32)
        for ci in range(8):
            n, dc = divmod(ci, 4)
            o4 = res.results[ci]["out"]  # (4, 64, 1024)
            out[n, :, 4 * dc : 4 * dc + DCHUNK] = o4.transpose(1, 0, 2).reshape(
                OC, DCHUNK, H, W
            )
        return out

    if ver == "v2":
        extra = _prep_v2_weights(K, bias, mnp)
        build = _build_nc_v2
        in_maps = []
        for ci in range(8):
            n, dc = divmod(ci, 4)
            slab = xpad[n, :, 4 * dc : 4 * dc + DSLAB].reshape(IC, SLABF)
            xs = np.zeros((IC, XS_COLS), mnp)
            xs[:, :SLABF] = slab.astype(mnp)
            in_maps.append({"xs": xs, **extra})
        _last_in_maps = in_maps
        _last_mm = mm
        _last_build = build
        nc = build(mm)
        res = run_bass_kernel_spmd(nc, in_maps, core_ids=list(range(8)))
        out = np.empty((N, OC, D, H, W), np.float32)
        for ci in range(8):
            n, dc = divmod(ci, 4)
            out[n, :, 4 * dc : 4 * dc + DCHUNK] = res.results[ci]["out"].reshape(
                OC, DCHUNK, H, W
            )
        return out

    if ver == "v0":
        kt = np.ascontiguousarray(KtF[:4].reshape(128, NTAPS_LJ * OC).astype(mnp))
        kt4 = np.ascontiguousarray(KtF[4].reshape(IC, NTAPS_LJ * OC).astype(mnp))
        extra = {"kt": kt, "kt4": kt4}
        build = _build_nc
    else:
        kta = np.ascontiguousarray(
            KtF[:4][:, :, LJ_A, :].reshape(128, len(LJ_A) * OC).astype(mnp)
        )
        ktb = np.ascontiguousarray(
            KtF[:4][:, :, LJ_B, :].reshape(128, len(LJ_B) * OC).astype(mnp)
        )
        # ktd: partition (l, ic) for l=0..3, free (j, oc): taps (l, j, i=4)
        ktd = np.zeros((128, 5 * OC), mnp)
        for j in range(5):
            for l in range(4):
                ktd[32 * l : 32 * (l + 1), j * OC : (j + 1) * OC] = KtF[
                    4, :, l * 5 + j, :
                ].astype(mnp)
        # ktj: partition (j, ic) for j=0..3: taps (l=4, j, i=4)
        ktj = np.zeros((128, OC), mnp)
        for j in range(4):
            ktj[32 * j : 32 * (j + 1), :] = KtF[4, :, 4 * 5 + j, :].astype(mnp)
        kt5 = np.ascontiguousarray(KtF[4, :, 24, :].astype(mnp))  # (l=4,j=4,i=4)
        extra = {"kta": kta, "ktb": ktb, "ktd": ktd, "ktj": ktj, "kt5": kt5}
        build = _build_nc_packed

    in_maps = []
    for ci in range(8):
        n, dc = divmod(ci, 4)
        slab = xpad[n, :, 4 * dc : 4 * dc + DSLAB].reshape(IC, SLABF)
        xs = np.zeros((IC, XS_COLS), mnp)
        xs[:, :SLABF] = slab.astype(mnp)
        in_maps.append({"xs": xs, "bias": bias_in, **extra})

    _last_in_maps = in_maps
    _last_mm = mm
    _last_build = build
    nc = build(mm)
    res = run_bass_kernel_spmd(nc, in_maps, core_ids=list(range(8)))

    out = np.empty((N, OC, D, H, W), np.float32)
    for ci in range(8):
        n, dc = divmod(ci, 4)
        out[n, :, 4 * dc : 4 * dc + DCHUNK] = res.results[ci]["out"].reshape(
            OC, DCHUNK, H, W
        )
    return out

